# revision 12
# baseline (speedup 1.0000x reference)
"""Trainium2 Bass kernel: AttentionAggregator (GNN message passing).

Reference math per branch (user / item):
    cat  = concat_k [ tabA[adjA[:,k]] | tabB[adjB[:,k]] ]      # [NJ, 256]
    S    = (q @ q.T) / sqrt(D) + 1                             # [NJ, NJ]
    agg  = softmax(S, axis=-1) @ cat                           # [NJ, 256]
    out  = relu(agg @ W)                                       # [NJ, 64]

Refactorings:
  * (softmax(S) @ cat) @ W == softmax(S) @ (cat @ W), and cat @ W decomposes
    per neighbor stream: VW[j] = sum_pr TW_pr[adj_pr[j]] where
    TW_pr = tab_pr @ W_slice_pr is HOST-precomputed ([N, 64] fp32 tables).
    The device only gathers TW rows and sums them (no on-device transpose
    or projection GEMM).
  * Scores are bounded so softmax needs no max subtraction:
    row = exp(S) @ [VW | 1]; the final normalize relu(row[:64]/row[64]).T
    happens on HOST (trivial elementwise postprocessing).

Sharding: 8 cores, row-parallel. Cores 0-3 take 2048-row slices of the user
branch, cores 4-7 of the item branch; one SPMD program, different data.
Each core gathers VW rows for its NJ/4 = 2048-row j-shard; shards are
exchanged with a per-chunk AllGather within each 4-core branch group.

Per-core dataflow (engines in parentheses):
  phase A:  dma_gather 8 TW streams x [1024, 64] fp32 per chunk (GpSimd)
            -> 7-way tree add (DVE) -> VW1 [128, 8, 65] bf16 -> DRAM
            -> AllGather (cc) -> vw_c [128, 32, 65] in SBUF
  runway:   S/exp for the first RW j-tiles of m-block 0 (PE/ACT), keeping
            both engines busy while phase A runs.
  main:     per (m-block, j-tile): S^T = qT_t.T @ qmT (2x512 MMs, fp32
            PSUM) -> E = exp(S/sqrt(D)+1) bf16 (ACT) -> out_ps[65, 1024]
            += VW1_t.T @ E (2x512 MMs).  G matmuls are emitted lazily
            (2 per round) so the PE FIFO never blocks waiting for VW.
  out:      out_ps -> SBUF (DVE) -> DRAM [65, M] fp32, normalized on host.
"""

import os
import sys

sys.path.insert(0, "/opt/trn_rl_repo")
os.environ.setdefault("MYCRO_LOCAL_CACHE", "1")

from collections import deque

import numpy as np

import concourse.bass as bass
import concourse.bacc as bacc
import concourse.mybir as mybir
import concourse.tile as tile

try:  # ml_dtypes ships with jax
    import ml_dtypes

    BF16_NP = ml_dtypes.bfloat16
except ImportError:  # pragma: no cover
    BF16_NP = None

P = 128


class Cfg:
    def __init__(self, NJ=8192, M=2048, NA=16384, NB=8192, D=32, K=4, OUT=64,
                 MBLK=1024, CH=1024, GSH=4, RW=66, EB=68, DRAIN=2):
        self.NJ = NJ      # attention length (rows of the branch)
        self.M = M        # rows this core owns
        self.NA = NA      # table A rows (review_vecs)
        self.NB = NB      # table B rows (item/user vecs)
        self.D = D        # embedding dim (contraction for scores)
        self.K = K        # neighbors per adjacency list
        self.OUT = OUT    # output dim
        self.MBLK = MBLK  # m-block width per exp/psum tile
        self.CH = CH      # gather chunk (j rows per gather round)
        self.GSH = GSH    # cores per branch group sharing the gather
        self.RW = RW      # runway rounds (S/exp emitted before any G)
        self.EB = EB      # exp pool buffers (E-tile backlog capacity)
        self.NPAIR = 2 * K            # gather streams
        self.JT = NJ // P             # j-tiles
        self.NMB = M // MBLK          # m-blocks
        self.CT = CH // P             # j-tiles per gather chunk
        self.JSH = NJ // GSH          # j rows gathered by this core
        self.SHT = self.JSH // P      # j-tiles in this core's shard
        self.NCH = self.JSH // CH     # gather chunks (local)
        assert NJ % P == 0 and M % MBLK == 0 and CH % P == 0
        assert self.JSH % CH == 0 and self.JSH % 16 == 0


def build_nc(cfg: Cfg) -> bass.Bass:
    NJ, M, NA, NB, D, K, OUT = cfg.NJ, cfg.M, cfg.NA, cfg.NB, cfg.D, cfg.K, cfg.OUT
    MBLK, CH, JT, NMB, CT, NCH = cfg.MBLK, cfg.CH, cfg.JT, cfg.NMB, cfg.CT, cfg.NCH
    GSH, JSH, SHT, NPAIR = cfg.GSH, cfg.JSH, cfg.SHT, cfg.NPAIR
    RW, EB = cfg.RW, cfg.EB
    O1 = OUT + 1
    bf16 = mybir.dt.bfloat16
    fp32 = mybir.dt.float32
    i16 = mybir.dt.int16

    nc = bacc.Bacc(num_devices=2 * GSH, num_swdge_queues=4)
    groups = [list(range(GSH)), list(range(GSH, 2 * GSH))]

    qT = nc.declare_dram_parameter("qT", [D, NJ], bf16, isOutput=False)
    qmT = nc.declare_dram_parameter("qmT", [D, M], bf16, isOutput=False)
    # host-projected neighbor tables: TWA[k] = tabA @ W[k*2D : k*2D+D],
    # TWB[k] = tabB @ W[k*2D+D : (k+1)*2D]  (fp32, 64 cols = 256B rows)
    TWA = nc.declare_dram_parameter("TWA", [K, NA, OUT], fp32, isOutput=False)
    TWB = nc.declare_dram_parameter("TWB", [K, NB, OUT], fp32, isOutput=False)
    # int16 indices for THIS core's j-shard, 16-partition-wrapped and
    # replicated across Q7 cores: idx[pair, p, s] = adj[js0 + s*16 + p%16, k]
    idx = nc.declare_dram_parameter("idx", [NPAIR, P, JSH // 16], i16,
                                    isOutput=False)
    # un-normalized output: rows 0..63 = exp(S) @ VW, row 64 = exp(S) @ 1
    out = nc.declare_dram_parameter("out", [O1, M], fp32, isOutput=True)

    # VW shard exchange buffers (chunked AllGather over the branch group)
    vw_shard_dram = nc.dram_tensor("vw_shard", [SHT, P, O1], bf16)
    vw_full_dram = [nc.dram_tensor(f"vw_full{c}", [GSH * CT, P, O1], bf16)
                    for c in range(NCH)]
    # j-tile processing order: chunk-c tiles of every rank come before
    # chunk-c+1 tiles, matching chunked-AllGather availability.
    t_order = [r * SHT + c * CT + i
               for c in range(NCH) for r in range(GSH) for i in range(CT)]
    assert sorted(t_order) == list(range(JT))

    inv_sqrt_d = 1.0 / float(np.sqrt(D))
    NHALF = 2
    HB = MBLK // NHALF  # 512 cols per matmul (one PSUM bank)

    with tile.TileContext(nc) as tc:
        with (
            tc.tile_pool(name="const", bufs=1) as const_pool,
            tc.tile_pool(name="gat", bufs=4) as gat_pool,
            tc.tile_pool(name="add", bufs=5) as add_pool,
            tc.tile_pool(name="vw1", bufs=2) as vw1_pool,
            tc.tile_pool(name="vwc", bufs=NCH) as vwc_pool,
            tc.tile_pool(name="exp", bufs=EB) as exp_pool,
            tc.tile_pool(name="osb", bufs=2) as osb_pool,
            tc.tile_pool(name="sps", bufs=2, space="PSUM") as sps_pool,
            tc.tile_pool(name="ops", bufs=NMB, space="PSUM") as ops_pool,
        ):
            # ---- constants / persistent SBUF tensors -----------------------
            idx_sb = const_pool.tile([P, NPAIR, JSH // 16], i16, tag="idx_sb")
            for pr in range(NPAIR):
                nc.sync.dma_start(out=idx_sb[:, pr, :], in_=idx[pr, :, :])

            # split the q loads so the first runway tiles are ready ASAP
            qT_sb = const_pool.tile([P, NJ], bf16, tag="qT_sb")
            qmT_sb = const_pool.tile([P, M], bf16, tag="qmT_sb")
            Q0 = 1024
            nc.sync.dma_start(out=qT_sb[0:D, 0:Q0], in_=qT[:, 0:Q0])
            nc.sync.dma_start(out=qmT_sb[0:D, 0:MBLK], in_=qmT[:, 0:MBLK])
            nc.sync.dma_start(out=qT_sb[0:D, Q0:NJ], in_=qT[:, Q0:NJ])
            nc.sync.dma_start(out=qmT_sb[0:D, MBLK:M], in_=qmT[:, MBLK:M])

            bias1 = const_pool.tile([P, 1], fp32, tag="bias1")
            nc.vector.memset(bias1[:], 1.0)

            # Warm-up Exp so the ACT table-set pseudo-load lands on an
            # instruction with few sync waits (walrus limit: 2 per inst),
            # not on the first pipelined exp of the main loop.
            warm = const_pool.tile([P, 1], fp32, tag="warm")
            nc.scalar.activation(
                out=warm[:], in_=bias1[:],
                func=mybir.ActivationFunctionType.Exp,
                bias=bias1[:, 0:1], scale=1.0)

            # ---- phase A: gather TW rows for this core's j-shard -----------
            # (GpSimd + DVE + DMA + cc only; runs concurrently with the
            # PE/ACT runway emitted below)
            vw_sbs = []
            qnum = 0
            for c in range(NCH):
                ic0 = c * (CH // 16)
                ic1 = (c + 1) * (CH // 16)
                sums = []  # pairwise-sum tree, interleaved with gathers
                prev_g = None
                for pr in range(NPAIR):
                    gat = gat_pool.tile([P, CT, OUT], fp32, tag="gat")
                    tw_src = TWA[pr, :, :] if pr < K else TWB[pr - K, :, :]
                    nc.gpsimd.dma_gather(
                        gat[:],
                        tw_src,
                        idx_sb[:, pr, ic0:ic1],
                        CH,
                        CH,
                        OUT,
                        queue_num=qnum % 4,
                    )
                    qnum += 1
                    if prev_g is None:
                        prev_g = gat
                    else:
                        a = add_pool.tile([P, CT, OUT], fp32, tag="a")
                        nc.vector.tensor_add(out=a[:], in0=prev_g[:], in1=gat[:])
                        sums.append(a)
                        prev_g = None
                while len(sums) > 1:
                    a = add_pool.tile([P, CT, OUT], fp32, tag="a")
                    nc.vector.tensor_add(out=a[:], in0=sums[0][:], in1=sums[1][:])
                    sums = sums[2:] + [a]
                vw1 = vw1_pool.tile([P, CT, O1], bf16, tag="vw1")
                nc.vector.tensor_copy(out=vw1[:, :, 0:OUT], in_=sums[0][:])
                nc.vector.memset(vw1[:, :, OUT:O1], 1.0)
                nc.sync.dma_start(
                    out=vw_shard_dram[c * CT:(c + 1) * CT, :, :].rearrange(
                        "t p c -> p t c"),
                    in_=vw1[:],
                )
                # AllGather this chunk immediately; high_priority nudges the
                # trigger ahead of the next chunk's gathers in the GpSimd
                # queue so the exchange overlaps them.
                with tc.high_priority():
                    nc.gpsimd.collective_compute(
                        "AllGather",
                        mybir.AluOpType.bypass,
                        replica_groups=groups,
                        ins=[vw_shard_dram[c * CT:(c + 1) * CT, :, :]],
                        outs=[vw_full_dram[c][:, :, :]],
                    )
                vw_c = vwc_pool.tile([P, GSH * CT, O1], bf16, tag="vw_c")
                nc.sync.dma_start(
                    out=vw_c[:],
                    in_=vw_full_dram[c][:, :, :].rearrange("t p c -> p t c"),
                )
                vw_sbs.append(vw_c)

            # ---- S/exp emission helper ------------------------------------
            e_store = {}

            def emit_sx(mb, s):
                t = t_order[s]
                s_ps = sps_pool.tile([P, MBLK], fp32, tag="s_ps")
                for h in range(NHALF):
                    nc.tensor.matmul(
                        out=s_ps[:, h * HB:(h + 1) * HB],
                        lhsT=qT_sb[0:D, t * P:(t + 1) * P],
                        rhs=qmT_sb[0:D,
                                   mb * MBLK + h * HB:mb * MBLK + (h + 1) * HB],
                        start=True,
                        stop=True,
                    )
                e_sb = exp_pool.tile([P, MBLK], bf16, tag="e_sb")
                nc.scalar.activation(
                    out=e_sb[:],
                    in_=s_ps[:],
                    func=mybir.ActivationFunctionType.Exp,
                    bias=bias1[:, 0:1],
                    scale=inv_sqrt_d,
                )
                e_store[(mb, s)] = e_sb

            out_pss = {}
            osbs = {}

            def emit_g(mb, s):
                if mb not in out_pss:
                    out_psn = ops_pool.tile([O1, MBLK], fp32, tag="out_ps")
                    out_pss[mb] = out_psn
                e_sb = e_store.pop((mb, s))
                vw_c = vw_sbs[s // (GSH * CT)]
                for h in range(NHALF):
                    nc.tensor.matmul(
                        out=out_pss[mb][:, h * HB:(h + 1) * HB],
                        lhsT=vw_c[:, s % (GSH * CT), :],
                        rhs=e_sb[:, h * HB:(h + 1) * HB],
                        start=(s == 0),
                        stop=(s == JT - 1),
                        skip_group_check=True,
                    )
                if s == JT - 1:  # m-block finished: stage + store
                    o_sb = osb_pool.tile([O1, MBLK], fp32, tag="o_sb")
                    nc.vector.tensor_copy(out=o_sb[:], in_=out_pss[mb][:])
                    osbs[mb] = o_sb
                    nc.sync.dma_start(
                        out=out[:, mb * MBLK:(mb + 1) * MBLK], in_=o_sb[:])

            # ---- runway: pure S/exp for the first RW rounds ---------------
            # ACT runs these back-to-back while phase A completes; the E
            # tiles stack up in the exp pool (EB >= RW).
            rounds = [(mb, s) for mb in range(NMB) for s in range(JT)]
            for (mb, s) in rounds[:RW]:
                emit_sx(mb, s)

            # ---- G-block: dense burst of the runway's G matmuls -----------
            # One contiguous dependency-free PE stretch (only gated on VW
            # availability, chunk-0 tiles first) — warms the PE clock ramp.
            for (mb, s) in rounds[:RW]:
                emit_g(mb, s)

            # ---- steady rounds: S/exp + lag-1 G ---------------------------
            for ri in range(RW, len(rounds)):
                emit_sx(*rounds[ri])
                if ri > RW:
                    emit_g(*rounds[ri - 1])
            emit_g(*rounds[-1])

    nc.finalize()
    return nc


# --------------------------------------------------------------------------
# host side
# --------------------------------------------------------------------------

def _bf16(a: np.ndarray) -> np.ndarray:
    return np.ascontiguousarray(a.astype(BF16_NP))


def _wrap_idx(arr: np.ndarray) -> np.ndarray:
    # [NJ] -> [128, NJ//16] int16: idx position i = s*16 + p%16 at [p, s],
    # replicated across the eight 16-partition Q7 groups
    w = arr.astype(np.int16).reshape(-1, 16).T  # [16, NJ//16]
    return np.ascontiguousarray(np.tile(w, (P // 16, 1)))


def _make_idx(adjA: np.ndarray, adjB: np.ndarray, K: int) -> np.ndarray:
    streams = [_wrap_idx(adjA[:, k]) for k in range(K)]
    streams += [_wrap_idx(adjB[:, k]) for k in range(K)]
    return np.ascontiguousarray(np.stack(streams, axis=0))


def _make_tw(tabA: np.ndarray, tabB: np.ndarray, W: np.ndarray,
             K: int, D: int) -> tuple[np.ndarray, np.ndarray]:
    # TWA[k] = tabA @ W[k*2D : k*2D+D];  TWB[k] = tabB @ W[k*2D+D : (k+1)*2D]
    f32 = np.float32
    TWA = np.stack([tabA.astype(f32) @ W[k * 2 * D:k * 2 * D + D].astype(f32)
                    for k in range(K)])
    TWB = np.stack([tabB.astype(f32) @ W[k * 2 * D + D:(k + 1) * 2 * D].astype(f32)
                    for k in range(K)])
    return np.ascontiguousarray(TWA), np.ascontiguousarray(TWB)


def _make_in_maps(cfg: Cfg, review_vecs, user_vecs, item_vecs,
                  adj_ur, adj_ri, adj_ir, adj_ru,
                  user_neigh_W, item_neigh_W, n_cores=8):
    half = n_cores // 2
    uT = _bf16(np.asarray(user_vecs).T)
    iT = _bf16(np.asarray(item_vecs).T)
    idx_u = _make_idx(np.asarray(adj_ur), np.asarray(adj_ri), cfg.K)
    idx_i = _make_idx(np.asarray(adj_ir), np.asarray(adj_ru), cfg.K)
    TWA_u, TWB_u = _make_tw(np.asarray(review_vecs), np.asarray(item_vecs),
                            np.asarray(user_neigh_W), cfg.K, cfg.D)
    TWA_i, TWB_i = _make_tw(np.asarray(review_vecs), np.asarray(user_vecs),
                            np.asarray(item_neigh_W), cfg.K, cfg.D)
    SW = cfg.JSH // 16  # wrapped-index columns per shard

    in_maps = []
    for core in range(n_cores):
        if core < half:  # user branch
            qTb, ix, twa, twb = uT, idx_u, TWA_u, TWB_u
        else:  # item branch
            qTb, ix, twa, twb = iT, idx_i, TWA_i, TWB_i
        s = (core % half) * cfg.M
        sh = core % cfg.GSH
        in_maps.append({
            "qT": qTb,
            "qmT": np.ascontiguousarray(qTb[:, s:s + cfg.M]),
            "TWA": twa,
            "TWB": twb,
            "idx": np.ascontiguousarray(ix[:, :, sh * SW:(sh + 1) * SW]),
        })
    return in_maps


_BUILT = {}


def _get_nc(cfg: Cfg) -> bass.Bass:
    key = tuple(sorted(cfg.__dict__.items()))
    if key not in _BUILT:
        _BUILT[key] = build_nc(cfg)
    return _BUILT[key]


def kernel(review_vecs, user_vecs, item_vecs, adj_ur, adj_ri, adj_ir, adj_ru,
           user_neigh_W, item_neigh_W, _trace=False):
    from concourse.bass_utils import run_bass_kernel_spmd

    n_cores = 8
    Nu = np.asarray(user_vecs).shape[0]
    Ni = np.asarray(item_vecs).shape[0]
    cfg = Cfg(NJ=Nu, M=Nu // (n_cores // 2),
              NA=np.asarray(review_vecs).shape[0], NB=Ni)
    nc = _get_nc(cfg)
    in_maps = _make_in_maps(cfg, review_vecs, user_vecs, item_vecs,
                            adj_ur, adj_ri, adj_ir, adj_ru,
                            user_neigh_W, item_neigh_W, n_cores)
    res = run_bass_kernel_spmd(nc, in_maps, core_ids=list(range(n_cores)),
                               trace=_trace)
    # host epilogue: out rows 0..63 = exp(S) @ VW, row 64 = denominator
    branch = []
    for i in range(n_cores):
        o = np.asarray(res.results[i]["out"], dtype=np.float32)  # [65, M]
        branch.append(np.maximum(o[:64] / o[64:65], 0.0).T)      # [M, 64]
    user_out = np.concatenate(branch[: n_cores // 2], axis=0)
    item_out = np.concatenate(branch[n_cores // 2:], axis=0)
    if _trace:
        return (user_out, item_out), res
    return user_out, item_out


# revision 16
# speedup vs baseline: 1.4414x; 1.4414x over previous
"""Trainium2 Bass kernel: AttentionAggregator (GNN message passing).

Reference math per branch (user / item):
    cat  = concat_k [ tabA[adjA[:,k]] | tabB[adjB[:,k]] ]      # [NJ, 256]
    S    = (q @ q.T) / sqrt(D) + 1                             # [NJ, NJ]
    agg  = softmax(S, axis=-1) @ cat                           # [NJ, 256]
    out  = relu(agg @ W)                                       # [NJ, 64]

Refactorings:
  * (softmax(S) @ cat) @ W == softmax(S) @ (cat @ W), and cat @ W decomposes
    per neighbor stream: VW[j] = sum_pr TW_pr[adj_pr[j]] where
    TW_pr = tab_pr @ W_slice_pr is HOST-precomputed ([N, 64] fp32 tables).
    The device only gathers TW rows and sums them (no on-device transpose
    or projection GEMM).
  * Scores are bounded so softmax needs no max subtraction:
    row = exp(S) @ [VW | 1]; the final normalize relu(row[:64]/row[64]).T
    happens on HOST (trivial elementwise postprocessing).

Sharding: 8 cores, row-parallel. Cores 0-3 take 2048-row slices of the user
branch, cores 4-7 of the item branch; one SPMD program, different data.
Each core gathers VW rows for its NJ/4 = 2048-row j-shard; shards are
exchanged with a per-chunk AllGather within each 4-core branch group.

Per-core dataflow (engines in parentheses):
  phase A:  dma_gather 8 TW streams x [1024, 64] fp32 per chunk (GpSimd)
            -> 7-way tree add (DVE) -> VW1 [128, 8, 65] bf16 -> DRAM
            -> AllGather (cc) -> vw_c [128, 32, 65] in SBUF
  runway:   S/exp for the first RW j-tiles of m-block 0 (PE/ACT), keeping
            both engines busy while phase A runs.
  main:     per (m-block, j-tile): S^T = qT_t.T @ qmT (2x512 MMs, fp32
            PSUM) -> E = exp(S/sqrt(D)+1) bf16 (ACT) -> out_ps[65, 1024]
            += VW1_t.T @ E (2x512 MMs).  G matmuls are emitted lazily
            (2 per round) so the PE FIFO never blocks waiting for VW.
  out:      out_ps -> SBUF (DVE) -> DRAM [65, M] fp32, normalized on host.
"""

import os
import sys

sys.path.insert(0, "/opt/trn_rl_repo")
os.environ.setdefault("MYCRO_LOCAL_CACHE", "1")

from collections import deque

import numpy as np

import concourse.bass as bass
import concourse.bacc as bacc
import concourse.mybir as mybir
import concourse.tile as tile

try:  # ml_dtypes ships with jax
    import ml_dtypes

    BF16_NP = ml_dtypes.bfloat16
except ImportError:  # pragma: no cover
    BF16_NP = None

P = 128


class Cfg:
    def __init__(self, NJ=8192, M=2048, NA=16384, NB=8192, D=32, K=4, OUT=64,
                 MBLK=1024, CH=1024, GSH=4, RW=68, EB=69, DRAIN=2):
        self.NJ = NJ      # attention length (rows of the branch)
        self.M = M        # rows this core owns
        self.NA = NA      # table A rows (review_vecs)
        self.NB = NB      # table B rows (item/user vecs)
        self.D = D        # embedding dim (contraction for scores)
        self.K = K        # neighbors per adjacency list
        self.OUT = OUT    # output dim
        self.MBLK = MBLK  # m-block width per exp/psum tile
        self.CH = CH      # gather chunk (j rows per gather round)
        self.GSH = GSH    # cores per branch group sharing the gather
        self.RW = RW      # runway rounds (S/exp emitted before any G)
        self.EB = EB      # exp pool buffers (E-tile backlog capacity)
        self.NPAIR = 2 * K            # gather streams
        self.JT = NJ // P             # j-tiles
        self.NMB = M // MBLK          # m-blocks
        self.CT = CH // P             # j-tiles per gather chunk
        self.JSH = NJ // GSH          # j rows gathered by this core
        self.SHT = self.JSH // P      # j-tiles in this core's shard
        self.NCH = self.JSH // CH     # gather chunks (local)
        assert NJ % P == 0 and M % MBLK == 0 and CH % P == 0
        assert self.JSH % CH == 0 and self.JSH % 16 == 0


def build_nc(cfg: Cfg) -> bass.Bass:
    NJ, M, NA, NB, D, K, OUT = cfg.NJ, cfg.M, cfg.NA, cfg.NB, cfg.D, cfg.K, cfg.OUT
    MBLK, CH, JT, NMB, CT, NCH = cfg.MBLK, cfg.CH, cfg.JT, cfg.NMB, cfg.CT, cfg.NCH
    GSH, JSH, SHT, NPAIR = cfg.GSH, cfg.JSH, cfg.SHT, cfg.NPAIR
    RW, EB = cfg.RW, cfg.EB
    O1 = OUT + 1
    bf16 = mybir.dt.bfloat16
    fp32 = mybir.dt.float32
    i16 = mybir.dt.int16

    nc = bacc.Bacc(num_devices=2 * GSH, num_swdge_queues=4)
    groups = [list(range(GSH)), list(range(GSH, 2 * GSH))]

    qT = nc.declare_dram_parameter("qT", [D, NJ], bf16, isOutput=False)
    qmT = nc.declare_dram_parameter("qmT", [D, M], bf16, isOutput=False)
    # host-projected neighbor tables: TWA[k] = tabA @ W[k*2D : k*2D+D],
    # TWB[k] = tabB @ W[k*2D+D : (k+1)*2D]  (fp32, 64 cols = 256B rows)
    TWA = nc.declare_dram_parameter("TWA", [K, NA, OUT], fp32, isOutput=False)
    TWB = nc.declare_dram_parameter("TWB", [K, NB, OUT], fp32, isOutput=False)
    # int16 indices for THIS core's j-shard, 16-partition-wrapped and
    # replicated across Q7 cores: idx[pair, p, s] = adj[js0 + s*16 + p%16, k]
    idx = nc.declare_dram_parameter("idx", [NPAIR, P, JSH // 16], i16,
                                    isOutput=False)
    # un-normalized output: rows 0..63 = exp(S) @ VW, row 64 = exp(S) @ 1
    out = nc.declare_dram_parameter("out", [O1, M], fp32, isOutput=True)

    # VW shard exchange buffers (chunked AllGather over the branch group).
    # Partition-major layout so both the store and the gathered loads are
    # clean contiguous 2D DMAs (a strided rearrange here costs ~80us of
    # descriptor crawl on a contended queue).
    vw_shard_dram = [nc.dram_tensor(f"vw_shard{c}", [P, CT, O1], bf16)
                     for c in range(NCH)]
    vw_full_dram = [nc.dram_tensor(f"vw_full{c}", [GSH * P, CT, O1], bf16)
                    for c in range(NCH)]
    # j-tile processing order: chunk-c tiles of every rank come before
    # chunk-c+1 tiles, matching chunked-AllGather availability.
    t_order = [r * SHT + c * CT + i
               for c in range(NCH) for r in range(GSH) for i in range(CT)]
    assert sorted(t_order) == list(range(JT))

    inv_sqrt_d = 1.0 / float(np.sqrt(D))
    NHALF = 2
    HB = MBLK // NHALF  # 512 cols per matmul (one PSUM bank)

    with tile.TileContext(nc) as tc:
        with (
            tc.tile_pool(name="const", bufs=1) as const_pool,
            tc.tile_pool(name="gat", bufs=8) as gat_pool,
            tc.tile_pool(name="add", bufs=5) as add_pool,
            tc.tile_pool(name="vw1", bufs=2) as vw1_pool,
            tc.tile_pool(name="vwc", bufs=NCH) as vwc_pool,
            tc.tile_pool(name="exp", bufs=EB) as exp_pool,
            tc.tile_pool(name="osb", bufs=2) as osb_pool,
            tc.tile_pool(name="sps", bufs=2, space="PSUM") as sps_pool,
            tc.tile_pool(name="ops", bufs=NMB, space="PSUM") as ops_pool,
        ):
            # ---- constants / persistent SBUF tensors -----------------------
            idx_sb = const_pool.tile([P, NPAIR, JSH // 16], i16, tag="idx_sb")
            for pr in range(NPAIR):
                nc.sync.dma_start(out=idx_sb[:, pr, :], in_=idx[pr, :, :])

            # split the q loads so the first runway tiles are ready ASAP
            qT_sb = const_pool.tile([P, NJ], bf16, tag="qT_sb")
            qmT_sb = const_pool.tile([P, M], bf16, tag="qmT_sb")
            Q0 = 1024
            nc.sync.dma_start(out=qT_sb[0:D, 0:Q0], in_=qT[:, 0:Q0])
            nc.sync.dma_start(out=qmT_sb[0:D, 0:MBLK], in_=qmT[:, 0:MBLK])
            nc.sync.dma_start(out=qT_sb[0:D, Q0:NJ], in_=qT[:, Q0:NJ])
            nc.sync.dma_start(out=qmT_sb[0:D, MBLK:M], in_=qmT[:, MBLK:M])

            bias1 = const_pool.tile([P, 1], fp32, tag="bias1")
            nc.vector.memset(bias1[:], 1.0)

            # Warm-up Exp so the ACT table-set pseudo-load lands on an
            # instruction with few sync waits (walrus limit: 2 per inst),
            # not on the first pipelined exp of the main loop.
            warm = const_pool.tile([P, 1], fp32, tag="warm")
            nc.scalar.activation(
                out=warm[:], in_=bias1[:],
                func=mybir.ActivationFunctionType.Exp,
                bias=bias1[:, 0:1], scale=1.0)

            # ---- phase A: gather TW rows for this core's j-shard -----------
            # (GpSimd + DVE + DMA + cc only; runs concurrently with the
            # PE/ACT runway emitted below)
            vw_sbs = []
            qnum = 0
            for c in range(NCH):
                ic0 = c * (CH // 16)
                ic1 = (c + 1) * (CH // 16)
                sums = []  # pairwise-sum tree, interleaved with gathers
                prev_g = None
                for pr in range(NPAIR):
                    gat = gat_pool.tile([P, CT, OUT], fp32, tag="gat")
                    tw_src = TWA[pr, :, :] if pr < K else TWB[pr - K, :, :]
                    nc.gpsimd.dma_gather(
                        gat[:],
                        tw_src,
                        idx_sb[:, pr, ic0:ic1],
                        CH,
                        CH,
                        OUT,
                        queue_num=qnum % 4,
                    )
                    qnum += 1
                    if prev_g is None:
                        prev_g = gat
                    else:
                        a = add_pool.tile([P, CT, OUT], fp32, tag="a")
                        nc.vector.tensor_add(out=a[:], in0=prev_g[:], in1=gat[:])
                        sums.append(a)
                        prev_g = None
                while len(sums) > 1:
                    a = add_pool.tile([P, CT, OUT], fp32, tag="a")
                    nc.vector.tensor_add(out=a[:], in0=sums[0][:], in1=sums[1][:])
                    sums = sums[2:] + [a]
                vw1 = vw1_pool.tile([P, CT, O1], bf16, tag="vw1")
                nc.vector.tensor_copy(out=vw1[:, :, 0:OUT], in_=sums[0][:])
                nc.vector.memset(vw1[:, :, OUT:O1], 1.0)
                nc.sync.dma_start(out=vw_shard_dram[c][:, :, :], in_=vw1[:])
                nc.gpsimd.collective_compute(
                    "AllGather",
                    mybir.AluOpType.bypass,
                    replica_groups=groups,
                    ins=[vw_shard_dram[c][:, :, :]],
                    outs=[vw_full_dram[c][:, :, :]],
                )
                vw_c = vwc_pool.tile([P, GSH * CT, O1], bf16, tag="vw_c")
                for r in range(GSH):
                    nc.sync.dma_start(
                        out=vw_c[:, r * CT:(r + 1) * CT, :],
                        in_=vw_full_dram[c][r * P:(r + 1) * P, :, :],
                    )
                vw_sbs.append(vw_c)

            # ---- S/exp emission helper ------------------------------------
            e_store = {}

            def emit_sx(mb, s):
                t = t_order[s]
                s_ps = sps_pool.tile([P, MBLK], fp32, tag="s_ps")
                for h in range(NHALF):
                    nc.tensor.matmul(
                        out=s_ps[:, h * HB:(h + 1) * HB],
                        lhsT=qT_sb[0:D, t * P:(t + 1) * P],
                        rhs=qmT_sb[0:D,
                                   mb * MBLK + h * HB:mb * MBLK + (h + 1) * HB],
                        start=True,
                        stop=True,
                    )
                e_sb = exp_pool.tile([P, MBLK], bf16, tag="e_sb")
                nc.scalar.activation(
                    out=e_sb[:],
                    in_=s_ps[:],
                    func=mybir.ActivationFunctionType.Exp,
                    bias=bias1[:, 0:1],
                    scale=inv_sqrt_d,
                )
                e_store[(mb, s)] = e_sb

            out_pss = {}
            osbs = {}

            def emit_g(mb, s):
                if mb not in out_pss:
                    out_psn = ops_pool.tile([O1, MBLK], fp32, tag="out_ps")
                    out_pss[mb] = out_psn
                e_sb = e_store.pop((mb, s))
                vw_c = vw_sbs[s // (GSH * CT)]
                for h in range(NHALF):
                    nc.tensor.matmul(
                        out=out_pss[mb][:, h * HB:(h + 1) * HB],
                        lhsT=vw_c[:, s % (GSH * CT), :],
                        rhs=e_sb[:, h * HB:(h + 1) * HB],
                        start=(s == 0),
                        stop=(s == JT - 1),
                        skip_group_check=True,
                    )
                if s == JT - 1:  # m-block finished: stage + store
                    o_sb = osb_pool.tile([O1, MBLK], fp32, tag="o_sb")
                    nc.vector.tensor_copy(out=o_sb[:], in_=out_pss[mb][:])
                    osbs[mb] = o_sb
                    nc.sync.dma_start(
                        out=out[:, mb * MBLK:(mb + 1) * MBLK], in_=o_sb[:])

            # ---- runway: pure S/exp for the first RW rounds ---------------
            # ACT runs these back-to-back while phase A completes; the E
            # tiles stack up in the exp pool (EB >= RW).
            rounds = [(mb, s) for mb in range(NMB) for s in range(JT)]
            for (mb, s) in rounds[:RW]:
                emit_sx(mb, s)

            # ---- G-block: dense burst of the runway's G matmuls -----------
            # One contiguous dependency-free PE stretch (only gated on VW
            # availability, chunk-0 tiles first) — warms the PE clock ramp.
            for (mb, s) in rounds[:RW]:
                emit_g(mb, s)

            # ---- steady rounds: S/exp + lag-1 G ---------------------------
            for ri in range(RW, len(rounds)):
                emit_sx(*rounds[ri])
                if ri > RW:
                    emit_g(*rounds[ri - 1])
            emit_g(*rounds[-1])

    nc.finalize()
    return nc


# --------------------------------------------------------------------------
# host side
# --------------------------------------------------------------------------

def _bf16(a: np.ndarray) -> np.ndarray:
    return np.ascontiguousarray(a.astype(BF16_NP))


def _wrap_idx(arr: np.ndarray) -> np.ndarray:
    # [NJ] -> [128, NJ//16] int16: idx position i = s*16 + p%16 at [p, s],
    # replicated across the eight 16-partition Q7 groups
    w = arr.astype(np.int16).reshape(-1, 16).T  # [16, NJ//16]
    return np.ascontiguousarray(np.tile(w, (P // 16, 1)))


def _make_idx(adjA: np.ndarray, adjB: np.ndarray, K: int) -> np.ndarray:
    streams = [_wrap_idx(adjA[:, k]) for k in range(K)]
    streams += [_wrap_idx(adjB[:, k]) for k in range(K)]
    return np.ascontiguousarray(np.stack(streams, axis=0))


def _make_tw(tabA: np.ndarray, tabB: np.ndarray, W: np.ndarray,
             K: int, D: int) -> tuple[np.ndarray, np.ndarray]:
    # TWA[k] = tabA @ W[k*2D : k*2D+D];  TWB[k] = tabB @ W[k*2D+D : (k+1)*2D]
    f32 = np.float32
    TWA = np.stack([tabA.astype(f32) @ W[k * 2 * D:k * 2 * D + D].astype(f32)
                    for k in range(K)])
    TWB = np.stack([tabB.astype(f32) @ W[k * 2 * D + D:(k + 1) * 2 * D].astype(f32)
                    for k in range(K)])
    return np.ascontiguousarray(TWA), np.ascontiguousarray(TWB)


def _make_in_maps(cfg: Cfg, review_vecs, user_vecs, item_vecs,
                  adj_ur, adj_ri, adj_ir, adj_ru,
                  user_neigh_W, item_neigh_W, n_cores=8):
    half = n_cores // 2
    uT = _bf16(np.asarray(user_vecs).T)
    iT = _bf16(np.asarray(item_vecs).T)
    idx_u = _make_idx(np.asarray(adj_ur), np.asarray(adj_ri), cfg.K)
    idx_i = _make_idx(np.asarray(adj_ir), np.asarray(adj_ru), cfg.K)
    TWA_u, TWB_u = _make_tw(np.asarray(review_vecs), np.asarray(item_vecs),
                            np.asarray(user_neigh_W), cfg.K, cfg.D)
    TWA_i, TWB_i = _make_tw(np.asarray(review_vecs), np.asarray(user_vecs),
                            np.asarray(item_neigh_W), cfg.K, cfg.D)
    SW = cfg.JSH // 16  # wrapped-index columns per shard

    in_maps = []
    for core in range(n_cores):
        if core < half:  # user branch
            qTb, ix, twa, twb = uT, idx_u, TWA_u, TWB_u
        else:  # item branch
            qTb, ix, twa, twb = iT, idx_i, TWA_i, TWB_i
        s = (core % half) * cfg.M
        sh = core % cfg.GSH
        in_maps.append({
            "qT": qTb,
            "qmT": np.ascontiguousarray(qTb[:, s:s + cfg.M]),
            "TWA": twa,
            "TWB": twb,
            "idx": np.ascontiguousarray(ix[:, :, sh * SW:(sh + 1) * SW]),
        })
    return in_maps


_BUILT = {}


def _get_nc(cfg: Cfg) -> bass.Bass:
    key = tuple(sorted(cfg.__dict__.items()))
    if key not in _BUILT:
        _BUILT[key] = build_nc(cfg)
    return _BUILT[key]


def kernel(review_vecs, user_vecs, item_vecs, adj_ur, adj_ri, adj_ir, adj_ru,
           user_neigh_W, item_neigh_W, _trace=False):
    from concourse.bass_utils import run_bass_kernel_spmd

    n_cores = 8
    Nu = np.asarray(user_vecs).shape[0]
    Ni = np.asarray(item_vecs).shape[0]
    cfg = Cfg(NJ=Nu, M=Nu // (n_cores // 2),
              NA=np.asarray(review_vecs).shape[0], NB=Ni)
    nc = _get_nc(cfg)
    in_maps = _make_in_maps(cfg, review_vecs, user_vecs, item_vecs,
                            adj_ur, adj_ri, adj_ir, adj_ru,
                            user_neigh_W, item_neigh_W, n_cores)
    res = run_bass_kernel_spmd(nc, in_maps, core_ids=list(range(n_cores)),
                               trace=_trace)
    # host epilogue: out rows 0..63 = exp(S) @ VW, row 64 = denominator
    branch = []
    for i in range(n_cores):
        o = np.asarray(res.results[i]["out"], dtype=np.float32)  # [65, M]
        branch.append(np.maximum(o[:64] / o[64:65], 0.0).T)      # [M, 64]
    user_out = np.concatenate(branch[: n_cores // 2], axis=0)
    item_out = np.concatenate(branch[n_cores // 2:], axis=0)
    if _trace:
        return (user_out, item_out), res
    return user_out, item_out


# revision 17
# speedup vs baseline: 1.7228x; 1.1952x over previous
"""Trainium2 Bass kernel: AttentionAggregator (GNN message passing).

Reference math per branch (user / item):
    cat  = concat_k [ tabA[adjA[:,k]] | tabB[adjB[:,k]] ]      # [NJ, 256]
    S    = (q @ q.T) / sqrt(D) + 1                             # [NJ, NJ]
    agg  = softmax(S, axis=-1) @ cat                           # [NJ, 256]
    out  = relu(agg @ W)                                       # [NJ, 64]

Refactorings:
  * (softmax(S) @ cat) @ W == softmax(S) @ (cat @ W), and cat @ W decomposes
    per neighbor stream: VW[j] = sum_pr TW_pr[adj_pr[j]] where
    TW_pr = tab_pr @ W_slice_pr is HOST-precomputed ([N, 64] fp32 tables).
    The device only gathers TW rows and sums them (no on-device transpose
    or projection GEMM).
  * Scores are bounded so softmax needs no max subtraction:
    row = exp(S) @ [VW | 1]; the final normalize relu(row[:64]/row[64]).T
    happens on HOST (trivial elementwise postprocessing).

Sharding: 8 cores, row-parallel. Cores 0-3 take 2048-row slices of the user
branch, cores 4-7 of the item branch; one SPMD program, different data.
Each core gathers VW rows for its NJ/4 = 2048-row j-shard; shards are
exchanged with a per-chunk AllGather within each 4-core branch group.

Per-core dataflow (engines in parentheses):
  phase A:  dma_gather 8 TW streams x [1024, 64] fp32 per chunk (GpSimd)
            -> 7-way tree add (DVE) -> VW1 [128, 8, 65] bf16 -> DRAM
            -> AllGather (cc) -> vw_c [128, 32, 65] in SBUF
  runway:   S/exp for the first RW j-tiles of m-block 0 (PE/ACT), keeping
            both engines busy while phase A runs.
  main:     per (m-block, j-tile): S^T = qT_t.T @ qmT (2x512 MMs, fp32
            PSUM) -> E = exp(S/sqrt(D)+1) bf16 (ACT) -> out_ps[65, 1024]
            += VW1_t.T @ E (2x512 MMs).  G matmuls are emitted lazily
            (2 per round) so the PE FIFO never blocks waiting for VW.
  out:      out_ps -> SBUF (DVE) -> DRAM [65, M] fp32, normalized on host.
"""

import os
import sys

sys.path.insert(0, "/opt/trn_rl_repo")
os.environ.setdefault("MYCRO_LOCAL_CACHE", "1")

from collections import deque

import numpy as np

import concourse.bass as bass
import concourse.bacc as bacc
import concourse.mybir as mybir
import concourse.tile as tile

try:  # ml_dtypes ships with jax
    import ml_dtypes

    BF16_NP = ml_dtypes.bfloat16
except ImportError:  # pragma: no cover
    BF16_NP = None

P = 128


class Cfg:
    def __init__(self, NJ=8192, M=2048, NA=16384, NB=8192, D=32, K=4, OUT=64,
                 MBLK=1024, CH=1024, GSH=4, RW=68, EB=69, DRAIN=2):
        self.NJ = NJ      # attention length (rows of the branch)
        self.M = M        # rows this core owns
        self.NA = NA      # table A rows (review_vecs)
        self.NB = NB      # table B rows (item/user vecs)
        self.D = D        # embedding dim (contraction for scores)
        self.K = K        # neighbors per adjacency list
        self.OUT = OUT    # output dim
        self.MBLK = MBLK  # m-block width per exp/psum tile
        self.CH = CH      # gather chunk (j rows per gather round)
        self.GSH = GSH    # cores per branch group sharing the gather
        self.RW = RW      # runway rounds (S/exp emitted before any G)
        self.EB = EB      # exp pool buffers (E-tile backlog capacity)
        self.NPAIR = 2 * K            # gather streams
        self.JT = NJ // P             # j-tiles
        self.NMB = M // MBLK          # m-blocks
        self.CT = CH // P             # j-tiles per gather chunk
        self.JSH = NJ // GSH          # j rows gathered by this core
        self.SHT = self.JSH // P      # j-tiles in this core's shard
        self.NCH = self.JSH // CH     # gather chunks (local)
        assert NJ % P == 0 and M % MBLK == 0 and CH % P == 0
        assert self.JSH % CH == 0 and self.JSH % 16 == 0


def build_nc(cfg: Cfg) -> bass.Bass:
    NJ, M, NA, NB, D, K, OUT = cfg.NJ, cfg.M, cfg.NA, cfg.NB, cfg.D, cfg.K, cfg.OUT
    MBLK, CH, JT, NMB, CT, NCH = cfg.MBLK, cfg.CH, cfg.JT, cfg.NMB, cfg.CT, cfg.NCH
    GSH, JSH, SHT, NPAIR = cfg.GSH, cfg.JSH, cfg.SHT, cfg.NPAIR
    RW, EB = cfg.RW, cfg.EB
    O1 = OUT + 1
    bf16 = mybir.dt.bfloat16
    fp32 = mybir.dt.float32
    i16 = mybir.dt.int16

    nc = bacc.Bacc(num_devices=2 * GSH, num_swdge_queues=4)
    groups = [list(range(GSH)), list(range(GSH, 2 * GSH))]

    qT = nc.declare_dram_parameter("qT", [D, NJ], bf16, isOutput=False)
    qmT = nc.declare_dram_parameter("qmT", [D, M], bf16, isOutput=False)
    # host-projected neighbor tables: TWA[k] = tabA @ W[k*2D : k*2D+D],
    # TWB[k] = tabB @ W[k*2D+D : (k+1)*2D]  (fp32, 64 cols = 256B rows)
    TWA = nc.declare_dram_parameter("TWA", [K, NA, OUT], fp32, isOutput=False)
    TWB = nc.declare_dram_parameter("TWB", [K, NB, OUT], fp32, isOutput=False)
    # int16 indices for THIS core's j-shard, 16-partition-wrapped and
    # replicated across Q7 cores: idx[pair, p, s] = adj[js0 + s*16 + p%16, k]
    idx = nc.declare_dram_parameter("idx", [NPAIR, P, JSH // 16], i16,
                                    isOutput=False)
    # un-normalized output: rows 0..63 = exp(S) @ VW, row 64 = exp(S) @ 1
    out = nc.declare_dram_parameter("out", [O1, M], fp32, isOutput=True)

    # VW shard exchange buffers (chunked AllGather over the branch group).
    # Partition-major layout so both the store and the gathered loads are
    # clean contiguous 2D DMAs (a strided rearrange here costs ~80us of
    # descriptor crawl on a contended queue).
    vw_shard_dram = [nc.dram_tensor(f"vw_shard{c}", [P, CT, O1], bf16)
                     for c in range(NCH)]
    vw_full_dram = [nc.dram_tensor(f"vw_full{c}", [GSH * P, CT, O1], bf16)
                    for c in range(NCH)]
    # j-tile processing order: chunk-c tiles of every rank come before
    # chunk-c+1 tiles, matching chunked-AllGather availability.
    t_order = [r * SHT + c * CT + i
               for c in range(NCH) for r in range(GSH) for i in range(CT)]
    assert sorted(t_order) == list(range(JT))

    inv_sqrt_d = 1.0 / float(np.sqrt(D))
    NHALF = 2
    HB = MBLK // NHALF  # 512 cols per matmul (one PSUM bank)

    with tile.TileContext(nc) as tc:
        with (
            tc.tile_pool(name="const", bufs=1) as const_pool,
            tc.tile_pool(name="gat", bufs=8) as gat_pool,
            tc.tile_pool(name="add", bufs=5) as add_pool,
            tc.tile_pool(name="vw1", bufs=2) as vw1_pool,
            tc.tile_pool(name="vwc", bufs=NCH) as vwc_pool,
            tc.tile_pool(name="exp", bufs=EB) as exp_pool,
            tc.tile_pool(name="osb", bufs=2) as osb_pool,
            tc.tile_pool(name="sps", bufs=3, space="PSUM") as sps_pool,
            tc.tile_pool(name="ops", bufs=1, space="PSUM") as ops_pool,
        ):
            # ---- constants / persistent SBUF tensors -----------------------
            idx_sb = const_pool.tile([P, NPAIR, JSH // 16], i16, tag="idx_sb")
            for pr in range(NPAIR):
                nc.sync.dma_start(out=idx_sb[:, pr, :], in_=idx[pr, :, :])

            # split the q loads so the first runway tiles are ready ASAP
            qT_sb = const_pool.tile([P, NJ], bf16, tag="qT_sb")
            qmT_sb = const_pool.tile([P, M], bf16, tag="qmT_sb")
            Q0 = 1024
            nc.sync.dma_start(out=qT_sb[0:D, 0:Q0], in_=qT[:, 0:Q0])
            nc.sync.dma_start(out=qmT_sb[0:D, 0:MBLK], in_=qmT[:, 0:MBLK])
            nc.sync.dma_start(out=qT_sb[0:D, Q0:NJ], in_=qT[:, Q0:NJ])
            nc.sync.dma_start(out=qmT_sb[0:D, MBLK:M], in_=qmT[:, MBLK:M])

            bias1 = const_pool.tile([P, 1], fp32, tag="bias1")
            nc.vector.memset(bias1[:], 1.0)

            # Warm-up Exp so the ACT table-set pseudo-load lands on an
            # instruction with few sync waits (walrus limit: 2 per inst),
            # not on the first pipelined exp of the main loop.
            warm = const_pool.tile([P, 1], fp32, tag="warm")
            nc.scalar.activation(
                out=warm[:], in_=bias1[:],
                func=mybir.ActivationFunctionType.Exp,
                bias=bias1[:, 0:1], scale=1.0)

            # ---- phase A: gather TW rows for this core's j-shard -----------
            # (GpSimd + DVE + DMA + cc only; runs concurrently with the
            # PE/ACT runway emitted below)
            vw_sbs = []
            qnum = 0
            for c in range(NCH):
                ic0 = c * (CH // 16)
                ic1 = (c + 1) * (CH // 16)
                sums = []  # pairwise-sum tree, interleaved with gathers
                prev_g = None
                for pr in range(NPAIR):
                    gat = gat_pool.tile([P, CT, OUT], fp32, tag="gat")
                    tw_src = TWA[pr, :, :] if pr < K else TWB[pr - K, :, :]
                    nc.gpsimd.dma_gather(
                        gat[:],
                        tw_src,
                        idx_sb[:, pr, ic0:ic1],
                        CH,
                        CH,
                        OUT,
                        queue_num=qnum % 4,
                    )
                    qnum += 1
                    if prev_g is None:
                        prev_g = gat
                    else:
                        a = add_pool.tile([P, CT, OUT], fp32, tag="a")
                        nc.vector.tensor_add(out=a[:], in0=prev_g[:], in1=gat[:])
                        sums.append(a)
                        prev_g = None
                while len(sums) > 1:
                    a = add_pool.tile([P, CT, OUT], fp32, tag="a")
                    nc.vector.tensor_add(out=a[:], in0=sums[0][:], in1=sums[1][:])
                    sums = sums[2:] + [a]
                vw1 = vw1_pool.tile([P, CT, O1], bf16, tag="vw1")
                nc.vector.tensor_copy(out=vw1[:, :, 0:OUT], in_=sums[0][:])
                nc.vector.memset(vw1[:, :, OUT:O1], 1.0)
                nc.sync.dma_start(out=vw_shard_dram[c][:, :, :], in_=vw1[:])
                nc.gpsimd.collective_compute(
                    "AllGather",
                    mybir.AluOpType.bypass,
                    replica_groups=groups,
                    ins=[vw_shard_dram[c][:, :, :]],
                    outs=[vw_full_dram[c][:, :, :]],
                )
                vw_c = vwc_pool.tile([P, GSH * CT, O1], bf16, tag="vw_c")
                for r in range(GSH):
                    nc.sync.dma_start(
                        out=vw_c[:, r * CT:(r + 1) * CT, :],
                        in_=vw_full_dram[c][r * P:(r + 1) * P, :, :],
                    )
                vw_sbs.append(vw_c)

            # ---- S/exp emission helper ------------------------------------
            e_store = {}

            def emit_sx(mb, s):
                t = t_order[s]
                s_ps = sps_pool.tile([P, MBLK], fp32, tag="s_ps")
                for h in range(NHALF):
                    nc.tensor.matmul(
                        out=s_ps[:, h * HB:(h + 1) * HB],
                        lhsT=qT_sb[0:D, t * P:(t + 1) * P],
                        rhs=qmT_sb[0:D,
                                   mb * MBLK + h * HB:mb * MBLK + (h + 1) * HB],
                        start=True,
                        stop=True,
                    )
                e_sb = exp_pool.tile([P, MBLK], bf16, tag="e_sb")
                nc.scalar.activation(
                    out=e_sb[:],
                    in_=s_ps[:],
                    func=mybir.ActivationFunctionType.Exp,
                    bias=bias1[:, 0:1],
                    scale=inv_sqrt_d,
                )
                e_store[(mb, s)] = e_sb

            out_pss = {}
            osbs = {}

            def emit_g(mb, s):
                if mb not in out_pss:
                    out_psn = ops_pool.tile([O1, MBLK], fp32, tag="out_ps")
                    out_pss[mb] = out_psn
                e_sb = e_store.pop((mb, s))
                vw_c = vw_sbs[s // (GSH * CT)]
                for h in range(NHALF):
                    nc.tensor.matmul(
                        out=out_pss[mb][:, h * HB:(h + 1) * HB],
                        lhsT=vw_c[:, s % (GSH * CT), :],
                        rhs=e_sb[:, h * HB:(h + 1) * HB],
                        start=(s == 0),
                        stop=(s == JT - 1),
                        skip_group_check=True,
                    )
                if s == JT - 1:  # m-block finished: stage + store
                    o_sb = osb_pool.tile([O1, MBLK], fp32, tag="o_sb")
                    nc.vector.tensor_copy(out=o_sb[:], in_=out_pss[mb][:])
                    osbs[mb] = o_sb
                    nc.sync.dma_start(
                        out=out[:, mb * MBLK:(mb + 1) * MBLK], in_=o_sb[:])

            # ---- runway: pure S/exp for the first RW rounds ---------------
            # ACT runs these back-to-back while phase A completes; the E
            # tiles stack up in the exp pool (EB >= RW).
            rounds = [(mb, s) for mb in range(NMB) for s in range(JT)]
            for (mb, s) in rounds[:RW]:
                emit_sx(mb, s)

            # ---- G-block: dense burst of the runway's G matmuls -----------
            # One contiguous dependency-free PE stretch (only gated on VW
            # availability, chunk-0 tiles first) — warms the PE clock ramp.
            for (mb, s) in rounds[:RW]:
                emit_g(mb, s)

            # ---- steady rounds: S/exp + lag-1 G ---------------------------
            for ri in range(RW, len(rounds)):
                emit_sx(*rounds[ri])
                if ri > RW:
                    emit_g(*rounds[ri - 1])
            emit_g(*rounds[-1])

    nc.finalize()
    return nc


# --------------------------------------------------------------------------
# host side
# --------------------------------------------------------------------------

def _bf16(a: np.ndarray) -> np.ndarray:
    return np.ascontiguousarray(a.astype(BF16_NP))


def _wrap_idx(arr: np.ndarray) -> np.ndarray:
    # [NJ] -> [128, NJ//16] int16: idx position i = s*16 + p%16 at [p, s],
    # replicated across the eight 16-partition Q7 groups
    w = arr.astype(np.int16).reshape(-1, 16).T  # [16, NJ//16]
    return np.ascontiguousarray(np.tile(w, (P // 16, 1)))


def _make_idx(adjA: np.ndarray, adjB: np.ndarray, K: int) -> np.ndarray:
    streams = [_wrap_idx(adjA[:, k]) for k in range(K)]
    streams += [_wrap_idx(adjB[:, k]) for k in range(K)]
    return np.ascontiguousarray(np.stack(streams, axis=0))


def _make_tw(tabA: np.ndarray, tabB: np.ndarray, W: np.ndarray,
             K: int, D: int) -> tuple[np.ndarray, np.ndarray]:
    # TWA[k] = tabA @ W[k*2D : k*2D+D];  TWB[k] = tabB @ W[k*2D+D : (k+1)*2D]
    f32 = np.float32
    TWA = np.stack([tabA.astype(f32) @ W[k * 2 * D:k * 2 * D + D].astype(f32)
                    for k in range(K)])
    TWB = np.stack([tabB.astype(f32) @ W[k * 2 * D + D:(k + 1) * 2 * D].astype(f32)
                    for k in range(K)])
    return np.ascontiguousarray(TWA), np.ascontiguousarray(TWB)


def _make_in_maps(cfg: Cfg, review_vecs, user_vecs, item_vecs,
                  adj_ur, adj_ri, adj_ir, adj_ru,
                  user_neigh_W, item_neigh_W, n_cores=8):
    half = n_cores // 2
    uT = _bf16(np.asarray(user_vecs).T)
    iT = _bf16(np.asarray(item_vecs).T)
    idx_u = _make_idx(np.asarray(adj_ur), np.asarray(adj_ri), cfg.K)
    idx_i = _make_idx(np.asarray(adj_ir), np.asarray(adj_ru), cfg.K)
    TWA_u, TWB_u = _make_tw(np.asarray(review_vecs), np.asarray(item_vecs),
                            np.asarray(user_neigh_W), cfg.K, cfg.D)
    TWA_i, TWB_i = _make_tw(np.asarray(review_vecs), np.asarray(user_vecs),
                            np.asarray(item_neigh_W), cfg.K, cfg.D)
    SW = cfg.JSH // 16  # wrapped-index columns per shard

    in_maps = []
    for core in range(n_cores):
        if core < half:  # user branch
            qTb, ix, twa, twb = uT, idx_u, TWA_u, TWB_u
        else:  # item branch
            qTb, ix, twa, twb = iT, idx_i, TWA_i, TWB_i
        s = (core % half) * cfg.M
        sh = core % cfg.GSH
        in_maps.append({
            "qT": qTb,
            "qmT": np.ascontiguousarray(qTb[:, s:s + cfg.M]),
            "TWA": twa,
            "TWB": twb,
            "idx": np.ascontiguousarray(ix[:, :, sh * SW:(sh + 1) * SW]),
        })
    return in_maps


_BUILT = {}


def _get_nc(cfg: Cfg) -> bass.Bass:
    key = tuple(sorted(cfg.__dict__.items()))
    if key not in _BUILT:
        _BUILT[key] = build_nc(cfg)
    return _BUILT[key]


def kernel(review_vecs, user_vecs, item_vecs, adj_ur, adj_ri, adj_ir, adj_ru,
           user_neigh_W, item_neigh_W, _trace=False):
    from concourse.bass_utils import run_bass_kernel_spmd

    n_cores = 8
    Nu = np.asarray(user_vecs).shape[0]
    Ni = np.asarray(item_vecs).shape[0]
    cfg = Cfg(NJ=Nu, M=Nu // (n_cores // 2),
              NA=np.asarray(review_vecs).shape[0], NB=Ni)
    nc = _get_nc(cfg)
    in_maps = _make_in_maps(cfg, review_vecs, user_vecs, item_vecs,
                            adj_ur, adj_ri, adj_ir, adj_ru,
                            user_neigh_W, item_neigh_W, n_cores)
    res = run_bass_kernel_spmd(nc, in_maps, core_ids=list(range(n_cores)),
                               trace=_trace)
    # host epilogue: out rows 0..63 = exp(S) @ VW, row 64 = denominator
    branch = []
    for i in range(n_cores):
        o = np.asarray(res.results[i]["out"], dtype=np.float32)  # [65, M]
        branch.append(np.maximum(o[:64] / o[64:65], 0.0).T)      # [M, 64]
    user_out = np.concatenate(branch[: n_cores // 2], axis=0)
    item_out = np.concatenate(branch[n_cores // 2:], axis=0)
    if _trace:
        return (user_out, item_out), res
    return user_out, item_out
